# revision 15
# baseline (speedup 1.0000x reference)
"""Trainium2 Bass kernel for nn_BulkSpaceGenerator.

Math: the fast-marching scan g_k = g_{k-1} + (1/(k+1))(c_k - g_{k-1}) starting
from c_0 yields the running mean g_k = mean(c_0..c_k); the mean over k of those
is sum_j w_j c_j with w_j = (1/K)(H_K - H_j) (harmonic numbers). Since
c_j = tokens @ W[:, j*D:(j+1)*D] + b[j*D:(j+1)*D], the whole module is

    out = tokens @ W_eff + b_eff,   W_eff = sum_j w_j W_j,  b_eff = sum_j w_j b_j

The kernel folds W -> W_eff on-device and runs the (8192x1024)@(1024x1024)
matmul on the PE array, sharded over 8 cores as 4 feature-shards x 2
token-shards (minimizes per-core HBM traffic: ~13.6 MB in + 2 MB out).

Schedule (per core): one deadline-ordered HWDGE ring (sync) carries
[W-pair0 (2 halves), tokh0-pair0, ..., tokh1-pair0, tokh0-pair3, tokh1-rest]
so arrivals match consumption order. W is laid out j-major so each fold step
is one contiguous (128, 512) f16 DVE op (2x mode); folds are split across
DVE (pairs 0,2,3) and GPSIMD (pair 1 + bias). PSUM fits half the output
(8 banks of (128,512) f32 vs 16 (dt,mi) groups), so the matmul runs as two
waves over token halves, each kt-outer chasing the DMA stream. Evictions
(psum + bias -> f16) alternate ACT/DVE right as each group's accumulation
stops, so wave-2 banks free at ~2x eviction cadence; outputs leave via the
otherwise-idle ACT HWDGE ring as (128, 1024) f16 half-blocks.
"""

import os
from contextlib import ExitStack

import numpy as np

import concourse.bass as bass
import concourse.tile as tile
from concourse import bacc, mybir
from concourse.bass_utils import run_bass_kernel_spmd

D_MODEL = 1024
BULK_DIM = 10
B, N = 4, 2048
BN = B * N                     # 8192 tokens
NCORES = 8
F_SHARDS = 4                   # feature shards (d dimension)
T_SHARDS = 2                   # token shards
DS = D_MODEL // F_SHARDS       # 256 output features per core
MS = BN // T_SHARDS            # 4096 tokens per core
KT = D_MODEL // 128            # 8 contraction k-tiles
NPAIR = KT // 2                # 4 kt-pairs
DT = DS // 128                 # 2 output d-tiles of 128 per core
MCHUNK = 512
HALF_M = MS // 2               # 2048 tokens per wave
MI_W = HALF_M // MCHUNK        # 4 m-chunks per wave

# w_j = (1/K) * (H_K - H_j), H_j = sum_{i=1..j} 1/i
_H = np.cumsum(1.0 / np.arange(1, BULK_DIM + 1))
W_COEF = ((_H[-1] - np.concatenate([[0.0], _H[:-1]])) / BULK_DIM).tolist()

MODE = os.environ.get("BULK_KERNEL_MODE", "v3")
N_PREWARM = int(os.environ.get("BULK_KERNEL_PREWARM", "34"))

_BUILD_CACHE = {}


def _build(mode: str) -> bass.Bass:
    f32 = mybir.dt.float32
    f16 = mybir.dt.float16

    nc = bacc.Bacc("TRN2", target_bir_lowering=False, debug=False,
                   num_devices=NCORES)
    # tok: [half, pair, 128, ktoff*2048+m] f16
    tokT = nc.dram_tensor("tokT", [2, NPAIR, 128, 2 * HALF_M], f16,
                          kind="ExternalInput").ap()
    # W: [pair, 128, j*512 + ktoff*256 + d] f16  (j-major for contiguous fold)
    wsl = nc.dram_tensor("wsl", [NPAIR, 128, BULK_DIM * 2 * DS], f16,
                         kind="ExternalInput").ap()
    # bias: [128, j*2+dt] f32
    bsl = nc.dram_tensor("bsl", [128, BULK_DIM * DT], f32,
                         kind="ExternalInput").ap()
    # out: [wave, dt, 128, m] f16
    outT = nc.dram_tensor("outT", [2, DT, 128, HALF_M], f16,
                          kind="ExternalOutput").ap()

    mult = mybir.AluOpType.mult
    add = mybir.AluOpType.add
    HJ = BULK_DIM // 2          # j-split point for the W half-DMAs

    with tile.TileContext(nc) as tc, ExitStack() as ctx:
        wr_pool = ctx.enter_context(tc.tile_pool(name="wr", bufs=NPAIR + 1))
        weff_pool = ctx.enter_context(tc.tile_pool(name="weff", bufs=NPAIR))
        tok_pool = ctx.enter_context(tc.tile_pool(name="tok", bufs=2 * NPAIR))
        small_pool = ctx.enter_context(tc.tile_pool(name="small", bufs=6))
        psum_pool = ctx.enter_context(
            tc.tile_pool(name="psum", bufs=8, space="PSUM"))
        out_pool = ctx.enter_context(tc.tile_pool(name="osb", bufs=4))

        # zero operands for PE-warming no-op matmuls (memset on POOL so the
        # DVE queue head stays free for the fold chain)
        zmm = small_pool.tile([128, 128], f16, tag="zmm")
        nc.gpsimd.memset(zmm[:], 0.0)
        zrhs = small_pool.tile([128, MCHUNK], f16, tag="zrhs")
        nc.gpsimd.memset(zrhs[:], 0.0)

        # ---- DMA issue, deadline order on one HWDGE ring (sync) ----
        wrs = []
        toks = [[None] * NPAIR, [None] * NPAIR]

        def tok_dma(half, i):
            tk = tok_pool.tile([128, 2, HALF_M], f16)
            nc.sync.dma_start(tk[:], tokT[half, i])
            toks[half][i] = tk

        def w_dma(i):
            wr = wr_pool.tile([128, BULK_DIM, 2 * DS], f16)
            nc.sync.dma_start(wr[:, 0:HJ, :], wsl[i][:, 0:HJ * 2 * DS])
            nc.sync.dma_start(wr[:, HJ:, :], wsl[i][:, HJ * 2 * DS:])
            wrs.append(wr)

        # W leads tokens slightly so folds (the serial DVE chain) never stall
        w_dma(0)
        w_dma(1)
        tok_dma(0, 0)
        w_dma(2)
        tok_dma(0, 1)
        w_dma(3)
        tok_dma(0, 2)
        tok_dma(0, 3)
        for i in range(NPAIR):
            tok_dma(1, i)

        # bias via SWDGE (tiny)
        bt = small_pool.tile([128, BULK_DIM, DT], f32, tag="bt")
        nc.gpsimd.dma_start(bt[:], bsl[:, :])

        # ---- fold W_eff per kt-pair, all on DVE ----
        # tensor_scalar_mul runs at 4x (168 ns / 512 f16), tensor_tensor at
        # 2x (427 ns), scalar_tensor_tensor only at 1x (653 ns) -- so scale
        # each j-block into a scratch tile, then a halves-tree of in-place
        # TT adds: 10 scales + 4 adds ~= 4.8 us/pair vs 6.5 us for STT chains.
        weffs = [None] * NPAIR
        tmp_pool = ctx.enter_context(tc.tile_pool(name="tmp", bufs=2))

        def fold_pair(i):
            tmp = tmp_pool.tile([128, BULK_DIM, 2 * DS], f16)
            for j in range(BULK_DIM):
                nc.vector.tensor_scalar_mul(
                    tmp[:, j], wrs[i][:, j], W_COEF[j])
            nc.vector.tensor_tensor(tmp[:, 0:5], tmp[:, 0:5], tmp[:, 5:10],
                                    add)
            nc.vector.tensor_tensor(tmp[:, 0:2], tmp[:, 0:2], tmp[:, 2:4],
                                    add)
            nc.vector.tensor_tensor(tmp[:, 0], tmp[:, 0], tmp[:, 1], add)
            we = weff_pool.tile([128, 2, DS], f16)
            nc.vector.tensor_tensor(we[:], tmp[:, 0], tmp[:, 4], add)
            weffs[i] = we

        fold_pair(0)
        fold_pair(1)
        # bias fold: 10 STT ops on (128, 2) f32
        be2 = small_pool.tile([128, DT], f32, tag="be")
        nc.vector.tensor_scalar_mul(be2[:], bt[:, 0], W_COEF[0])
        for j in range(1, BULK_DIM):
            nc.vector.scalar_tensor_tensor(
                be2[:], bt[:, j], W_COEF[j], be2[:], mult, add)
        fold_pair(2)
        fold_pair(3)

        # ---- prewarm: keep the PE HAM clock busy until real MMs start ----
        dummy_ps = psum_pool.tile([128, MCHUNK], f32, name="ps", tag="ps")
        for _ in range(N_PREWARM):
            nc.tensor.matmul(dummy_ps[:], lhsT=zmm[:], rhs=zrhs[:],
                             start=False, stop=False)

        def run_wave(wave, psums):
            for i in range(NPAIR - 1):
                for ktoff in range(2):
                    kt = 2 * i + ktoff
                    for dt_i in range(DT):
                        lhsT = weffs[i][:, ktoff, dt_i * 128:(dt_i + 1) * 128]
                        for mi in range(MI_W):
                            nc.tensor.matmul(
                                psums[dt_i][mi][:],
                                lhsT=lhsT,
                                rhs=toks[wave][i][:, ktoff,
                                                  mi * MCHUNK:(mi + 1) * MCHUNK],
                                start=(kt == 0), stop=False)
            # last pair: group-inner so each group's stop-MM (and thus its
            # eviction, freeing the PSUM bank) lands as early as possible
            i = NPAIR - 1
            for dt_i in range(DT):
                for mi in range(MI_W):
                    for ktoff in range(2):
                        lhsT = weffs[i][:, ktoff, dt_i * 128:(dt_i + 1) * 128]
                        nc.tensor.matmul(
                            psums[dt_i][mi][:],
                            lhsT=lhsT,
                            rhs=toks[wave][i][:, ktoff,
                                              mi * MCHUNK:(mi + 1) * MCHUNK],
                            start=False, stop=(ktoff == 1))

        def evict_wave(wave, psums):
            # stop-MM order is (dt0 mi0..3, dt1 mi0..3); evict in that order.
            # Wave-1: all on ACT (DVE is still folding). Wave-2: alternate
            # ACT/DVE so banks free at ~2x single-engine cadence.
            ots = {}
            for dt_i in range(DT):
                ots[dt_i] = out_pool.tile([128, HALF_M], f16,
                                          name="ot", tag="ot")
            k = 0
            for dt_i in range(DT):
                for mi in range(MI_W):
                    dst = ots[dt_i][:, mi * MCHUNK:(mi + 1) * MCHUNK]
                    if wave == 0 or k % 2 == 0:
                        nc.scalar.add(dst, psums[dt_i][mi][:],
                                      be2[:, dt_i:dt_i + 1])
                    else:
                        nc.vector.tensor_scalar_add(
                            dst, psums[dt_i][mi][:], be2[:, dt_i:dt_i + 1])
                    k += 1
                    if mi == 1:
                        nc.scalar.dma_start(
                            outT[wave, dt_i, :, 0:2 * MCHUNK],
                            ots[dt_i][:, 0:2 * MCHUNK])
                    elif mi == 2 and wave == 1 and dt_i == DT - 1:
                        nc.scalar.dma_start(
                            outT[wave, dt_i, :, 2 * MCHUNK:3 * MCHUNK],
                            ots[dt_i][:, 2 * MCHUNK:3 * MCHUNK])
                if wave == 1 and dt_i == DT - 1:
                    nc.scalar.dma_start(
                        outT[wave, dt_i, :, 3 * MCHUNK:],
                        ots[dt_i][:, 3 * MCHUNK:])
                else:
                    nc.scalar.dma_start(
                        outT[wave, dt_i, :, 2 * MCHUNK:],
                        ots[dt_i][:, 2 * MCHUNK:])

        # wave 1: first psum set, reuse dummy_ps as group (0,0)
        psA = [[None] * MI_W for _ in range(DT)]
        psA[0][0] = dummy_ps
        for dt_i in range(DT):
            for mi in range(MI_W):
                if psA[dt_i][mi] is None:
                    psA[dt_i][mi] = psum_pool.tile(
                        [128, MCHUNK], f32, name="ps", tag="ps")
        run_wave(0, psA)
        evict_wave(0, psA)

        # wave 2: second psum set (recycles wave-1 banks after eviction)
        psB = [[psum_pool.tile([128, MCHUNK], f32, name="ps", tag="ps")
                for _ in range(MI_W)] for _ in range(DT)]
        run_wave(1, psB)
        evict_wave(1, psB)

    nc.compile()
    return nc


def _get_nc(mode: str) -> bass.Bass:
    if mode not in _BUILD_CACHE:
        _BUILD_CACHE[mode] = _build(mode)
    return _BUILD_CACHE[mode]


def _make_in_maps(boundary_tokens, W_b2b, b_b2b, mode):
    tok = np.asarray(boundary_tokens, dtype=np.float32).reshape(BN, D_MODEL)
    tok16 = tok.astype(np.float16)
    W = np.asarray(W_b2b, dtype=np.float32).astype(np.float16).reshape(
        D_MODEL, BULK_DIM, D_MODEL)
    b = np.asarray(b_b2b, dtype=np.float32).reshape(BULK_DIM, D_MODEL)
    in_maps = []
    for c in range(NCORES):
        f, t = divmod(c, T_SHARDS)
        dsl = slice(f * DS, (f + 1) * DS)
        # W slice (1024, 10, 256) -> [pair, 128, j*512 + ktoff*256 + d]
        wslc = W[:, :, dsl].reshape(NPAIR, 2, 128, BULK_DIM, DS)
        wslc = np.ascontiguousarray(wslc.transpose(0, 2, 3, 1, 4)).reshape(
            NPAIR, 128, BULK_DIM * 2 * DS)
        # tokens slice (4096, 1024) -> [half, pair, 128, ktoff*2048 + m]
        ts = tok16[t * MS:(t + 1) * MS].T            # (1024, 4096) k-major
        ts = ts.reshape(NPAIR, 2, 128, 2, HALF_M)    # [pair, ktoff, p, half, m]
        ts = np.ascontiguousarray(ts.transpose(3, 0, 2, 1, 4)).reshape(
            2, NPAIR, 128, 2 * HALF_M)
        # bias slice (256, 10) -> [128, j*2 + dt]
        bs = b[:, dsl].T.reshape(DT, 128, BULK_DIM)  # [dt, p, j]
        bs = np.ascontiguousarray(bs.transpose(1, 2, 0)).reshape(
            128, BULK_DIM * DT)
        in_maps.append({"tokT": ts, "wsl": wslc, "bsl": bs})
    return in_maps


def _assemble(results):
    out = np.empty((BN, D_MODEL), dtype=np.float32)
    for c in range(NCORES):
        f, t = divmod(c, T_SHARDS)
        o = results[c]["outT"].astype(np.float32)
        for wave in range(2):
            for dt_i in range(DT):
                out[t * MS + wave * HALF_M:t * MS + (wave + 1) * HALF_M,
                    f * DS + dt_i * 128:f * DS + (dt_i + 1) * 128] = \
                    o[wave, dt_i].T
    return out.reshape(B, N, D_MODEL)


def run(boundary_tokens, W_b2b, b_b2b, mode=None, **spmd_kwargs):
    mode = mode or MODE
    nc = _get_nc(mode)
    in_maps = _make_in_maps(boundary_tokens, W_b2b, b_b2b, mode)
    res = run_bass_kernel_spmd(nc, in_maps, list(range(NCORES)), **spmd_kwargs)
    return _assemble(res.results), res


def kernel(boundary_tokens, W_b2b, b_b2b):
    out, _ = run(boundary_tokens, W_b2b, b_b2b)
    return out


# revision 18
# speedup vs baseline: 1.0295x; 1.0295x over previous
"""Trainium2 Bass kernel for nn_BulkSpaceGenerator.

Math: the fast-marching scan g_k = g_{k-1} + (1/(k+1))(c_k - g_{k-1}) starting
from c_0 yields the running mean g_k = mean(c_0..c_k); the mean over k of those
is sum_j w_j c_j with w_j = (1/K)(H_K - H_j) (harmonic numbers). Since
c_j = tokens @ W[:, j*D:(j+1)*D] + b[j*D:(j+1)*D], the whole module is

    out = tokens @ W_eff + b_eff,   W_eff = sum_j w_j W_j,  b_eff = sum_j w_j b_j

The kernel folds W -> W_eff on-device and runs the (8192x1024)@(1024x1024)
matmul on the PE array, sharded over 8 cores as 4 feature-shards x 2
token-shards (minimizes per-core HBM traffic: ~13.6 MB in + 2 MB out).

Schedule (per core): one deadline-ordered HWDGE ring (sync) carries
[W-pair0 (2 halves), tokh0-pair0, ..., tokh1-pair0, tokh0-pair3, tokh1-rest]
so arrivals match consumption order. W is laid out j-major so each fold step
is one contiguous (128, 512) f16 DVE op (2x mode); folds are split across
DVE (pairs 0,2,3) and GPSIMD (pair 1 + bias). PSUM fits half the output
(8 banks of (128,512) f32 vs 16 (dt,mi) groups), so the matmul runs as two
waves over token halves, each kt-outer chasing the DMA stream. Evictions
(psum + bias -> f16) alternate ACT/DVE right as each group's accumulation
stops, so wave-2 banks free at ~2x eviction cadence; outputs leave via the
otherwise-idle ACT HWDGE ring as (128, 1024) f16 half-blocks.
"""

import os
from contextlib import ExitStack

import numpy as np

import concourse.bass as bass
import concourse.tile as tile
from concourse import bacc, mybir
from concourse.bass_utils import run_bass_kernel_spmd

D_MODEL = 1024
BULK_DIM = 10
B, N = 4, 2048
BN = B * N                     # 8192 tokens
NCORES = 8
F_SHARDS = 4                   # feature shards (d dimension)
T_SHARDS = 2                   # token shards
DS = D_MODEL // F_SHARDS       # 256 output features per core
MS = BN // T_SHARDS            # 4096 tokens per core
KT = D_MODEL // 128            # 8 contraction k-tiles
NPAIR = KT // 2                # 4 kt-pairs
DT = DS // 128                 # 2 output d-tiles of 128 per core
MCHUNK = 512
HALF_M = MS // 2               # 2048 tokens per wave
MI_W = HALF_M // MCHUNK        # 4 m-chunks per wave

# w_j = (1/K) * (H_K - H_j), H_j = sum_{i=1..j} 1/i
_H = np.cumsum(1.0 / np.arange(1, BULK_DIM + 1))
W_COEF = ((_H[-1] - np.concatenate([[0.0], _H[:-1]])) / BULK_DIM).tolist()

MODE = os.environ.get("BULK_KERNEL_MODE", "v3")
N_PREWARM = int(os.environ.get("BULK_KERNEL_PREWARM", "28"))

_BUILD_CACHE = {}


def _build(mode: str) -> bass.Bass:
    f32 = mybir.dt.float32
    f16 = mybir.dt.float16

    nc = bacc.Bacc("TRN2", target_bir_lowering=False, debug=False,
                   num_devices=NCORES)
    # tok: [half, pair, 128, ktoff*2048+m] f16
    tokT = nc.dram_tensor("tokT", [2, NPAIR, 128, 2 * HALF_M], f16,
                          kind="ExternalInput").ap()
    # W: [pair, 128, j*512 + ktoff*256 + d] f16  (j-major for contiguous fold)
    wsl = nc.dram_tensor("wsl", [NPAIR, 128, BULK_DIM * 2 * DS], f16,
                         kind="ExternalInput").ap()
    # bias: [128, j*2+dt] f32
    bsl = nc.dram_tensor("bsl", [128, BULK_DIM * DT], f32,
                         kind="ExternalInput").ap()
    # out: [wave, dt, 128, m] f16
    outT = nc.dram_tensor("outT", [2, DT, 128, HALF_M], f16,
                          kind="ExternalOutput").ap()

    mult = mybir.AluOpType.mult
    add = mybir.AluOpType.add
    HJ = BULK_DIM // 2          # j-split point for the W half-DMAs

    with tile.TileContext(nc) as tc, ExitStack() as ctx:
        wr_pool = ctx.enter_context(tc.tile_pool(name="wr", bufs=NPAIR + 1))
        weff_pool = ctx.enter_context(tc.tile_pool(name="weff", bufs=NPAIR))
        tok_pool = ctx.enter_context(tc.tile_pool(name="tok", bufs=2 * NPAIR))
        small_pool = ctx.enter_context(tc.tile_pool(name="small", bufs=6))
        psum_pool = ctx.enter_context(
            tc.tile_pool(name="psum", bufs=8, space="PSUM"))
        out_pool = ctx.enter_context(tc.tile_pool(name="osb", bufs=4))

        # zero operands for PE-warming no-op matmuls (memset on POOL so the
        # DVE queue head stays free for the fold chain)
        zmm = small_pool.tile([128, 128], f16, tag="zmm")
        nc.gpsimd.memset(zmm[:], 0.0)
        zrhs = small_pool.tile([128, MCHUNK], f16, tag="zrhs")
        nc.gpsimd.memset(zrhs[:], 0.0)

        # ---- DMA issue, deadline order on one HWDGE ring (sync) ----
        wrs = []
        toks = [[None] * NPAIR, [None] * NPAIR]

        def tok_dma(half, i):
            tk = tok_pool.tile([128, 2, HALF_M], f16)
            nc.sync.dma_start(tk[:], tokT[half, i])
            toks[half][i] = tk

        def w_dma(i):
            wr = wr_pool.tile([128, BULK_DIM, 2 * DS], f16)
            nc.sync.dma_start(wr[:, 0:HJ, :], wsl[i][:, 0:HJ * 2 * DS])
            nc.sync.dma_start(wr[:, HJ:, :], wsl[i][:, HJ * 2 * DS:])
            wrs.append(wr)

        # W leads tokens slightly so folds (the serial DVE chain) never stall
        w_dma(0)
        w_dma(1)
        tok_dma(0, 0)
        w_dma(2)
        tok_dma(0, 1)
        w_dma(3)
        tok_dma(0, 2)
        tok_dma(0, 3)
        for i in range(NPAIR):
            tok_dma(1, i)

        # bias via SWDGE (tiny)
        bt = small_pool.tile([128, BULK_DIM, DT], f32, tag="bt")
        nc.gpsimd.dma_start(bt[:], bsl[:, :])

        # ---- fold W_eff per kt-pair, all on DVE ----
        # tensor_scalar_mul runs at 4x (168 ns / 512 f16), tensor_tensor at
        # 2x (427 ns), scalar_tensor_tensor only at 1x (653 ns) -- so scale
        # each j-block into a scratch tile, then a halves-tree of in-place
        # TT adds: 10 scales + 4 adds ~= 4.8 us/pair vs 6.5 us for STT chains.
        weffs = [None] * NPAIR
        tmp_pool = ctx.enter_context(tc.tile_pool(name="tmp", bufs=2))

        def fold_pair(i):
            tmp = tmp_pool.tile([128, BULK_DIM, 2 * DS], f16)
            for j in range(BULK_DIM):
                nc.vector.tensor_scalar_mul(
                    tmp[:, j], wrs[i][:, j], W_COEF[j])
            nc.vector.tensor_tensor(tmp[:, 0:5], tmp[:, 0:5], tmp[:, 5:10],
                                    add)
            nc.vector.tensor_tensor(tmp[:, 0:2], tmp[:, 0:2], tmp[:, 2:4],
                                    add)
            nc.vector.tensor_tensor(tmp[:, 0], tmp[:, 0], tmp[:, 1], add)
            we = weff_pool.tile([128, 2, DS], f16)
            nc.vector.tensor_tensor(we[:], tmp[:, 0], tmp[:, 4], add)
            weffs[i] = we

        fold_pair(0)
        fold_pair(1)
        # bias fold: 10 STT ops on (128, 2) f32
        be2 = small_pool.tile([128, DT], f32, tag="be")
        nc.vector.tensor_scalar_mul(be2[:], bt[:, 0], W_COEF[0])
        for j in range(1, BULK_DIM):
            nc.vector.scalar_tensor_tensor(
                be2[:], bt[:, j], W_COEF[j], be2[:], mult, add)
        fold_pair(2)
        fold_pair(3)

        # ---- prewarm: keep the PE HAM clock busy until real MMs start ----
        dummy_ps = psum_pool.tile([128, MCHUNK], f32, name="ps", tag="ps")
        for _ in range(N_PREWARM):
            nc.tensor.matmul(dummy_ps[:], lhsT=zmm[:], rhs=zrhs[:],
                             start=False, stop=False)

        def run_wave(wave, psums):
            for i in range(NPAIR):
                for ktoff in range(2):
                    kt = 2 * i + ktoff
                    for dt_i in range(DT):
                        lhsT = weffs[i][:, ktoff, dt_i * 128:(dt_i + 1) * 128]
                        for mi in range(MI_W):
                            nc.tensor.matmul(
                                psums[dt_i][mi][:],
                                lhsT=lhsT,
                                rhs=toks[wave][i][:, ktoff,
                                                  mi * MCHUNK:(mi + 1) * MCHUNK],
                                start=(kt == 0), stop=(kt == KT - 1))

        def evict_wave(wave, psums):
            # stop-MM order is (dt0 mi0..3, dt1 mi0..3); evict in that order.
            # Wave-1: all on ACT (DVE is still folding). Wave-2: alternate
            # ACT/DVE so banks free at ~2x single-engine cadence.
            ots = {}
            for dt_i in range(DT):
                ots[dt_i] = out_pool.tile([128, HALF_M], f16,
                                          name="ot", tag="ot")
            k = 0
            for dt_i in range(DT):
                for mi in range(MI_W):
                    dst = ots[dt_i][:, mi * MCHUNK:(mi + 1) * MCHUNK]
                    if wave == 0 or k % 2 == 0:
                        nc.scalar.add(dst, psums[dt_i][mi][:],
                                      be2[:, dt_i:dt_i + 1])
                    else:
                        nc.vector.tensor_scalar_add(
                            dst, psums[dt_i][mi][:], be2[:, dt_i:dt_i + 1])
                    k += 1
                    if mi == 1:
                        nc.scalar.dma_start(
                            outT[wave, dt_i, :, 0:2 * MCHUNK],
                            ots[dt_i][:, 0:2 * MCHUNK])
                nc.scalar.dma_start(
                    outT[wave, dt_i, :, 2 * MCHUNK:],
                    ots[dt_i][:, 2 * MCHUNK:])

        # wave 1: first psum set, reuse dummy_ps as group (0,0)
        psA = [[None] * MI_W for _ in range(DT)]
        psA[0][0] = dummy_ps
        for dt_i in range(DT):
            for mi in range(MI_W):
                if psA[dt_i][mi] is None:
                    psA[dt_i][mi] = psum_pool.tile(
                        [128, MCHUNK], f32, name="ps", tag="ps")
        run_wave(0, psA)
        evict_wave(0, psA)

        # wave 2: second psum set (recycles wave-1 banks after eviction)
        psB = [[psum_pool.tile([128, MCHUNK], f32, name="ps", tag="ps")
                for _ in range(MI_W)] for _ in range(DT)]
        run_wave(1, psB)
        evict_wave(1, psB)

    nc.compile()
    return nc


def _get_nc(mode: str) -> bass.Bass:
    if mode not in _BUILD_CACHE:
        _BUILD_CACHE[mode] = _build(mode)
    return _BUILD_CACHE[mode]


def _make_in_maps(boundary_tokens, W_b2b, b_b2b, mode):
    tok = np.asarray(boundary_tokens, dtype=np.float32).reshape(BN, D_MODEL)
    tok16 = tok.astype(np.float16)
    W = np.asarray(W_b2b, dtype=np.float32).astype(np.float16).reshape(
        D_MODEL, BULK_DIM, D_MODEL)
    b = np.asarray(b_b2b, dtype=np.float32).reshape(BULK_DIM, D_MODEL)
    in_maps = []
    for c in range(NCORES):
        f, t = divmod(c, T_SHARDS)
        dsl = slice(f * DS, (f + 1) * DS)
        # W slice (1024, 10, 256) -> [pair, 128, j*512 + ktoff*256 + d]
        wslc = W[:, :, dsl].reshape(NPAIR, 2, 128, BULK_DIM, DS)
        wslc = np.ascontiguousarray(wslc.transpose(0, 2, 3, 1, 4)).reshape(
            NPAIR, 128, BULK_DIM * 2 * DS)
        # tokens slice (4096, 1024) -> [half, pair, 128, ktoff*2048 + m]
        ts = tok16[t * MS:(t + 1) * MS].T            # (1024, 4096) k-major
        ts = ts.reshape(NPAIR, 2, 128, 2, HALF_M)    # [pair, ktoff, p, half, m]
        ts = np.ascontiguousarray(ts.transpose(3, 0, 2, 1, 4)).reshape(
            2, NPAIR, 128, 2 * HALF_M)
        # bias slice (256, 10) -> [128, j*2 + dt]
        bs = b[:, dsl].T.reshape(DT, 128, BULK_DIM)  # [dt, p, j]
        bs = np.ascontiguousarray(bs.transpose(1, 2, 0)).reshape(
            128, BULK_DIM * DT)
        in_maps.append({"tokT": ts, "wsl": wslc, "bsl": bs})
    return in_maps


def _assemble(results):
    out = np.empty((BN, D_MODEL), dtype=np.float32)
    for c in range(NCORES):
        f, t = divmod(c, T_SHARDS)
        o = results[c]["outT"].astype(np.float32)
        for wave in range(2):
            for dt_i in range(DT):
                out[t * MS + wave * HALF_M:t * MS + (wave + 1) * HALF_M,
                    f * DS + dt_i * 128:f * DS + (dt_i + 1) * 128] = \
                    o[wave, dt_i].T
    return out.reshape(B, N, D_MODEL)


def run(boundary_tokens, W_b2b, b_b2b, mode=None, **spmd_kwargs):
    mode = mode or MODE
    nc = _get_nc(mode)
    in_maps = _make_in_maps(boundary_tokens, W_b2b, b_b2b, mode)
    res = run_bass_kernel_spmd(nc, in_maps, list(range(NCORES)), **spmd_kwargs)
    return _assemble(res.results), res


def kernel(boundary_tokens, W_b2b, b_b2b):
    out, _ = run(boundary_tokens, W_b2b, b_b2b)
    return out


# revision 20
# speedup vs baseline: 1.0645x; 1.0340x over previous
"""Trainium2 Bass kernel for nn_BulkSpaceGenerator.

Math: the fast-marching scan g_k = g_{k-1} + (1/(k+1))(c_k - g_{k-1}) starting
from c_0 yields the running mean g_k = mean(c_0..c_k); the mean over k of those
is sum_j w_j c_j with w_j = (1/K)(H_K - H_j) (harmonic numbers). Since
c_j = tokens @ W[:, j*D:(j+1)*D] + b[j*D:(j+1)*D], the whole module is

    out = tokens @ W_eff + b_eff,   W_eff = sum_j w_j W_j,  b_eff = sum_j w_j b_j

The kernel folds W -> W_eff on-device and runs the (8192x1024)@(1024x1024)
matmul on the PE array, sharded over 8 cores as 4 feature-shards x 2
token-shards (minimizes per-core HBM traffic: ~13.6 MB in + 2 MB out).

Schedule (per core): one deadline-ordered HWDGE ring (sync) carries
[W-pair0 (2 halves), tokh0-pair0, ..., tokh1-pair0, tokh0-pair3, tokh1-rest]
so arrivals match consumption order. W is laid out j-major so each fold step
is one contiguous (128, 512) f16 DVE op (2x mode); folds are split across
DVE (pairs 0,2,3) and GPSIMD (pair 1 + bias). PSUM fits half the output
(8 banks of (128,512) f32 vs 16 (dt,mi) groups), so the matmul runs as two
waves over token halves, each kt-outer chasing the DMA stream. Evictions
(psum + bias -> f16) alternate ACT/DVE right as each group's accumulation
stops, so wave-2 banks free at ~2x eviction cadence; outputs leave via the
otherwise-idle ACT HWDGE ring as (128, 1024) f16 half-blocks.
"""

import os
from contextlib import ExitStack

import numpy as np

import concourse.bass as bass
import concourse.tile as tile
from concourse import bacc, mybir
from concourse.bass_utils import run_bass_kernel_spmd

D_MODEL = 1024
BULK_DIM = 10
B, N = 4, 2048
BN = B * N                     # 8192 tokens
NCORES = 8
F_SHARDS = 4                   # feature shards (d dimension)
T_SHARDS = 2                   # token shards
DS = D_MODEL // F_SHARDS       # 256 output features per core
MS = BN // T_SHARDS            # 4096 tokens per core
KT = D_MODEL // 128            # 8 contraction k-tiles
NPAIR = KT // 2                # 4 kt-pairs
DT = DS // 128                 # 2 output d-tiles of 128 per core
MCHUNK = 512
HALF_M = MS // 2               # 2048 tokens per wave
MI_W = HALF_M // MCHUNK        # 4 m-chunks per wave

# w_j = (1/K) * (H_K - H_j), H_j = sum_{i=1..j} 1/i
_H = np.cumsum(1.0 / np.arange(1, BULK_DIM + 1))
W_COEF = ((_H[-1] - np.concatenate([[0.0], _H[:-1]])) / BULK_DIM).tolist()

MODE = os.environ.get("BULK_KERNEL_MODE", "v3")
N_PREWARM = int(os.environ.get("BULK_KERNEL_PREWARM", "28"))

_BUILD_CACHE = {}


def _build(mode: str) -> bass.Bass:
    f32 = mybir.dt.float32
    f16 = mybir.dt.float16

    nc = bacc.Bacc("TRN2", target_bir_lowering=False, debug=False,
                   num_devices=NCORES)
    # tok: [half, pair, 128, ktoff*2048+m] f16
    tokT = nc.dram_tensor("tokT", [2, NPAIR, 128, 2 * HALF_M], f16,
                          kind="ExternalInput").ap()
    # W: [pair, 128, j*512 + ktoff*256 + d] f16  (j-major for contiguous fold)
    wsl = nc.dram_tensor("wsl", [NPAIR, 128, BULK_DIM * 2 * DS], f16,
                         kind="ExternalInput").ap()
    # bias: [128, j*2+dt] f32
    bsl = nc.dram_tensor("bsl", [128, BULK_DIM * DT], f32,
                         kind="ExternalInput").ap()
    # out: [wave, dt, 128, m] f16
    outT = nc.dram_tensor("outT", [2, DT, 128, HALF_M], f16,
                          kind="ExternalOutput").ap()

    mult = mybir.AluOpType.mult
    add = mybir.AluOpType.add
    HJ = BULK_DIM // 2          # j-split point for the W half-DMAs

    with tile.TileContext(nc) as tc, ExitStack() as ctx:
        wr_pool = ctx.enter_context(tc.tile_pool(name="wr", bufs=NPAIR + 1))
        weff_pool = ctx.enter_context(tc.tile_pool(name="weff", bufs=NPAIR))
        tok_pool = ctx.enter_context(tc.tile_pool(name="tok", bufs=2 * NPAIR))
        small_pool = ctx.enter_context(tc.tile_pool(name="small", bufs=6))
        psum_pool = ctx.enter_context(
            tc.tile_pool(name="psum", bufs=8, space="PSUM"))
        out_pool = ctx.enter_context(tc.tile_pool(name="osb", bufs=4))

        # zero operands for PE-warming no-op matmuls (memset on POOL so the
        # DVE queue head stays free for the fold chain)
        zmm = small_pool.tile([128, 128], f16, tag="zmm")
        nc.gpsimd.memset(zmm[:], 0.0)
        zrhs = small_pool.tile([128, MCHUNK], f16, tag="zrhs")
        nc.gpsimd.memset(zrhs[:], 0.0)

        # ---- DMA issue, deadline order on one HWDGE ring (sync) ----
        wrs = []
        toks = [[None] * NPAIR, [None] * NPAIR]

        def tok_dma(half, i):
            tk = tok_pool.tile([128, 2, HALF_M], f16)
            nc.sync.dma_start(tk[:], tokT[half, i])
            toks[half][i] = tk

        def w_dma(i):
            wr = wr_pool.tile([128, BULK_DIM, 2 * DS], f16)
            nc.sync.dma_start(wr[:, 0:HJ, :], wsl[i][:, 0:HJ * 2 * DS])
            nc.sync.dma_start(wr[:, HJ:, :], wsl[i][:, HJ * 2 * DS:])
            wrs.append(wr)

        # W leads tokens slightly so folds (the serial DVE chain) never stall
        w_dma(0)
        w_dma(1)
        tok_dma(0, 0)
        w_dma(2)
        tok_dma(0, 1)
        w_dma(3)
        tok_dma(0, 2)
        tok_dma(0, 3)
        for i in range(NPAIR):
            tok_dma(1, i)

        # bias via SWDGE (tiny)
        bt = small_pool.tile([128, BULK_DIM, DT], f32, tag="bt")
        nc.gpsimd.dma_start(bt[:], bsl[:, :])

        # ---- fold W_eff per kt-pair, all on DVE ----
        # tensor_scalar_mul runs at 4x (168 ns / 512 f16), tensor_tensor at
        # 2x (427 ns), scalar_tensor_tensor only at 1x (653 ns) -- so scale
        # each j-block into a scratch tile, then a halves-tree of in-place
        # TT adds: 10 scales + 4 adds ~= 4.8 us/pair vs 6.5 us for STT chains.
        weffs = [None] * NPAIR
        tmp_pool = ctx.enter_context(tc.tile_pool(name="tmp", bufs=2))

        def fold_pair(i):
            tmp = tmp_pool.tile([128, BULK_DIM, 2 * DS], f16)
            for j in range(BULK_DIM):
                nc.vector.tensor_scalar_mul(
                    tmp[:, j], wrs[i][:, j], W_COEF[j])
            nc.vector.tensor_tensor(tmp[:, 0:5], tmp[:, 0:5], tmp[:, 5:10],
                                    add)
            nc.vector.tensor_tensor(tmp[:, 0:2], tmp[:, 0:2], tmp[:, 2:4],
                                    add)
            nc.vector.tensor_tensor(tmp[:, 0], tmp[:, 0], tmp[:, 1], add)
            we = weff_pool.tile([128, 2, DS], f16)
            nc.vector.tensor_tensor(we[:], tmp[:, 0], tmp[:, 4], add)
            weffs[i] = we

        fold_pair(0)
        fold_pair(1)
        # bias fold: 10 STT ops on (128, 2) f32
        be2 = small_pool.tile([128, DT], f32, tag="be")
        nc.vector.tensor_scalar_mul(be2[:], bt[:, 0], W_COEF[0])
        for j in range(1, BULK_DIM):
            nc.vector.scalar_tensor_tensor(
                be2[:], bt[:, j], W_COEF[j], be2[:], mult, add)
        fold_pair(2)
        fold_pair(3)

        # ---- prewarm: keep the PE HAM clock busy until real MMs start ----
        dummy_ps = psum_pool.tile([128, MCHUNK], f32, name="ps", tag="ps")
        for _ in range(N_PREWARM):
            nc.tensor.matmul(dummy_ps[:], lhsT=zmm[:], rhs=zrhs[:],
                             start=False, stop=False)

        def run_wave(wave, psums):
            for i in range(NPAIR - 1):
                for ktoff in range(2):
                    kt = 2 * i + ktoff
                    for dt_i in range(DT):
                        lhsT = weffs[i][:, ktoff, dt_i * 128:(dt_i + 1) * 128]
                        for mi in range(MI_W):
                            nc.tensor.matmul(
                                psums[dt_i][mi][:],
                                lhsT=lhsT,
                                rhs=toks[wave][i][:, ktoff,
                                                  mi * MCHUNK:(mi + 1) * MCHUNK],
                                start=(kt == 0), stop=False)
            # last pair: group-inner so each group's stop-MM (hence its
            # eviction, freeing the PSUM bank) lands as early as possible
            i = NPAIR - 1
            for dt_i in range(DT):
                for mi in range(MI_W):
                    for ktoff in range(2):
                        nc.tensor.matmul(
                            psums[dt_i][mi][:],
                            lhsT=weffs[i][:, ktoff,
                                          dt_i * 128:(dt_i + 1) * 128],
                            rhs=toks[wave][i][:, ktoff,
                                              mi * MCHUNK:(mi + 1) * MCHUNK],
                            start=False, stop=(ktoff == 1))

        def evict_wave(wave, psums):
            # stop-MM order is (dt0 mi0..3, dt1 mi0..3); evict in that order,
            # alternating ACT/DVE so banks free at ~2x single-engine cadence
            # (the folds are done by the time wave-1's groups stop).
            ots = {}
            for dt_i in range(DT):
                ots[dt_i] = out_pool.tile([128, HALF_M], f16,
                                          name="ot", tag="ot")
            k = 0
            for dt_i in range(DT):
                for mi in range(MI_W):
                    dst = ots[dt_i][:, mi * MCHUNK:(mi + 1) * MCHUNK]
                    if k % 2 == 0:
                        nc.scalar.add(dst, psums[dt_i][mi][:],
                                      be2[:, dt_i:dt_i + 1])
                    else:
                        nc.vector.tensor_scalar_add(
                            dst, psums[dt_i][mi][:], be2[:, dt_i:dt_i + 1])
                    k += 1
                    if mi == 1:
                        nc.scalar.dma_start(
                            outT[wave, dt_i, :, 0:2 * MCHUNK],
                            ots[dt_i][:, 0:2 * MCHUNK])
                    elif mi == 2 and wave == 1 and dt_i == DT - 1:
                        # split the very last block so the final DMA is small
                        nc.scalar.dma_start(
                            outT[wave, dt_i, :, 2 * MCHUNK:3 * MCHUNK],
                            ots[dt_i][:, 2 * MCHUNK:3 * MCHUNK])
                if wave == 1 and dt_i == DT - 1:
                    nc.scalar.dma_start(
                        outT[wave, dt_i, :, 3 * MCHUNK:],
                        ots[dt_i][:, 3 * MCHUNK:])
                else:
                    nc.scalar.dma_start(
                        outT[wave, dt_i, :, 2 * MCHUNK:],
                        ots[dt_i][:, 2 * MCHUNK:])

        # wave 1: first psum set, reuse dummy_ps as group (0,0)
        psA = [[None] * MI_W for _ in range(DT)]
        psA[0][0] = dummy_ps
        for dt_i in range(DT):
            for mi in range(MI_W):
                if psA[dt_i][mi] is None:
                    psA[dt_i][mi] = psum_pool.tile(
                        [128, MCHUNK], f32, name="ps", tag="ps")
        run_wave(0, psA)
        evict_wave(0, psA)

        # wave 2: second psum set (recycles wave-1 banks after eviction)
        psB = [[psum_pool.tile([128, MCHUNK], f32, name="ps", tag="ps")
                for _ in range(MI_W)] for _ in range(DT)]
        run_wave(1, psB)
        evict_wave(1, psB)

    nc.compile()
    return nc


def _get_nc(mode: str) -> bass.Bass:
    if mode not in _BUILD_CACHE:
        _BUILD_CACHE[mode] = _build(mode)
    return _BUILD_CACHE[mode]


def _make_in_maps(boundary_tokens, W_b2b, b_b2b, mode):
    tok = np.asarray(boundary_tokens, dtype=np.float32).reshape(BN, D_MODEL)
    tok16 = tok.astype(np.float16)
    W = np.asarray(W_b2b, dtype=np.float32).astype(np.float16).reshape(
        D_MODEL, BULK_DIM, D_MODEL)
    b = np.asarray(b_b2b, dtype=np.float32).reshape(BULK_DIM, D_MODEL)
    in_maps = []
    for c in range(NCORES):
        f, t = divmod(c, T_SHARDS)
        dsl = slice(f * DS, (f + 1) * DS)
        # W slice (1024, 10, 256) -> [pair, 128, j*512 + ktoff*256 + d]
        wslc = W[:, :, dsl].reshape(NPAIR, 2, 128, BULK_DIM, DS)
        wslc = np.ascontiguousarray(wslc.transpose(0, 2, 3, 1, 4)).reshape(
            NPAIR, 128, BULK_DIM * 2 * DS)
        # tokens slice (4096, 1024) -> [half, pair, 128, ktoff*2048 + m]
        ts = tok16[t * MS:(t + 1) * MS].T            # (1024, 4096) k-major
        ts = ts.reshape(NPAIR, 2, 128, 2, HALF_M)    # [pair, ktoff, p, half, m]
        ts = np.ascontiguousarray(ts.transpose(3, 0, 2, 1, 4)).reshape(
            2, NPAIR, 128, 2 * HALF_M)
        # bias slice (256, 10) -> [128, j*2 + dt]
        bs = b[:, dsl].T.reshape(DT, 128, BULK_DIM)  # [dt, p, j]
        bs = np.ascontiguousarray(bs.transpose(1, 2, 0)).reshape(
            128, BULK_DIM * DT)
        in_maps.append({"tokT": ts, "wsl": wslc, "bsl": bs})
    return in_maps


def _assemble(results):
    out = np.empty((BN, D_MODEL), dtype=np.float32)
    for c in range(NCORES):
        f, t = divmod(c, T_SHARDS)
        o = results[c]["outT"].astype(np.float32)
        for wave in range(2):
            for dt_i in range(DT):
                out[t * MS + wave * HALF_M:t * MS + (wave + 1) * HALF_M,
                    f * DS + dt_i * 128:f * DS + (dt_i + 1) * 128] = \
                    o[wave, dt_i].T
    return out.reshape(B, N, D_MODEL)


def run(boundary_tokens, W_b2b, b_b2b, mode=None, **spmd_kwargs):
    mode = mode or MODE
    nc = _get_nc(mode)
    in_maps = _make_in_maps(boundary_tokens, W_b2b, b_b2b, mode)
    res = run_bass_kernel_spmd(nc, in_maps, list(range(NCORES)), **spmd_kwargs)
    return _assemble(res.results), res


def kernel(boundary_tokens, W_b2b, b_b2b):
    out, _ = run(boundary_tokens, W_b2b, b_b2b)
    return out
